# revision 33
# baseline (speedup 1.0000x reference)
"""Trainium2 Bass kernel for an 8-head attention layer (B=4, T=2048, K=512, H=8).

Sharding: DP=4 over batch x TP=2 over heads across 8 NeuronCores.
Core c handles batch c//2 with heads [4*(c%2), 4*(c%2)+4).
Each core emits its PARTIAL unify output for all 2048 tokens (f32); the
host sums the two partials of each pair and adds the bias (gather/unshard).

Matmul precision: projections / attention-output / unify run in bf16
(f32 PSUM). The attention-score matmuls run in fp8 (e4m3) DoubleRow mode
(2x PE throughput for that third of the FLOPs); qT/kT are stored with the
two 128-feature halves of each DoubleRow pair adjacent so both matmul
operands are single contiguous SBUF blocks. A x4 prescale on Wq/Wk
(undone by the exp activation's scale=1/16) keeps q/k out of the fp8
subnormal range. Softmax skips max-subtraction (scores bounded, |s|<~3).

Engine balance: exp + v-copies on Scalar, q/k-copies + softmax sums (bf16)
+ reciprocal_approx_fast + normalize on Vector. Scores for key-chunk kc+1
issue ahead of the output matmuls for chunk kc so exp latency is hidden.
"""

import os

import numpy as np
import ml_dtypes

# Number of leading 128-key chunks (of 16) per query block whose
# attention-output matmuls run in fp8 DoubleRow (rest bf16). Measured
# on HW: only ~5us faster at 4 (the DR pairs wait on both exps) while
# raising rel err 1.40e-2 -> 1.76e-2, so default off.
NF8 = int(os.environ.get("ATTNOUT_F8", "0"))

# Problem constants (hardcoded; kernel.py must be self-contained).
B, T, K, H = 4, 2048, 512, 8
NCORES = 8
HL = H // 2        # heads per core (TP=2)
DL = HL * K        # local concat feature dim = 2048
P = 128
KC = K // P        # input-dim chunks = 4
TC = T // P        # token chunks of 128 = 16
QB = 4             # query blocks of 512
PRE = 4.0          # fp8 prescale folded into Wq/Wk on the host

_NC_CACHE = {}


def _build_nc():
    import concourse.mybir as mybir
    import concourse.tile as tile
    from concourse import bacc

    f32 = mybir.dt.float32
    bf16 = mybir.dt.bfloat16
    f8 = mybir.dt.float8e4
    DR = mybir.MatmulPerfMode.DoubleRow
    Exp = mybir.ActivationFunctionType.Exp

    nc = bacc.Bacc("TRN2", target_bir_lowering=False, debug=False,
                   num_devices=NCORES)

    xT_d = nc.dram_tensor("xT", [K, T], bf16, kind="ExternalInput")
    wq_d = nc.dram_tensor("wq", [K, DL], bf16, kind="ExternalInput")
    wk_d = nc.dram_tensor("wk", [K, DL], bf16, kind="ExternalInput")
    wv_d = nc.dram_tensor("wv", [K, DL], bf16, kind="ExternalInput")
    wu_d = nc.dram_tensor("wu", [DL, K], bf16, kind="ExternalInput")
    out_d = nc.dram_tensor("out", [T, K], f32, kind="ExternalOutput")

    with tile.TileContext(nc) as tc:
        with (
            tc.tile_pool(name="const", bufs=1) as constp,
            tc.tile_pool(name="big", bufs=1) as bigp,
            tc.tile_pool(name="qkv", bufs=1) as qkvp,
            tc.tile_pool(name="attn", bufs=3) as attnp,
            tc.tile_pool(name="outp", bufs=1) as outp,
            tc.tile_pool(name="up", bufs=3) as up,
            tc.tile_pool(name="ps_mm", bufs=2, space="PSUM") as ps_mm,
            tc.tile_pool(name="ps_s", bufs=2, space="PSUM") as ps_s,
            tc.tile_pool(name="ps_o", bufs=4, space="PSUM") as ps_o,
        ):
            ones = constp.tile([P, P], bf16)
            nc.vector.memset(ones[:], 1.0)
            warm = constp.tile([P, 512], bf16)
            nc.vector.memset(warm[:], 0.0)
            # Warm the PE clock (HAM) during the initial DMA wait.
            for _ in range(16):
                wps = ps_mm.tile([P, 512], f32, tag="mm")
                nc.tensor.matmul(wps[:], ones[:, :], warm[:],
                                 start=True, stop=True)

            # All input DMAs issued up front, ordered so head 0's V
            # projection can start as early as possible; the first loads
            # are split per input-chunk to spread across DMA queues.
            wq_sb, wk_sb, wv_sb = [], [], []

            def load_w(lst, src_d, h, tag, split=False):
                t = constp.tile([P, KC, K], bf16, name=f"{tag}{h}")
                col = slice(h * K, (h + 1) * K)
                if split:
                    for kc in range(KC):
                        nc.sync.dma_start(
                            t[:, kc, :],
                            src_d.ap()[kc * P:(kc + 1) * P, col])
                else:
                    nc.sync.dma_start(
                        t[:], src_d.ap()[:, col].rearrange(
                            "(c p) d -> p c d", p=P))
                lst.append(t)

            load_w(wv_sb, wv_d, 0, "wv", split=True)
            xt = []
            for tb in range(QB):
                t = bigp.tile([P, KC, 512], bf16, name=f"xt{tb}")
                tsl = slice(tb * 512, (tb + 1) * 512)
                if tb == 0:
                    for kc in range(KC):
                        nc.sync.dma_start(
                            t[:, kc, :],
                            xT_d.ap()[kc * P:(kc + 1) * P, tsl])
                else:
                    nc.sync.dma_start(
                        t[:], xT_d.ap()[:, tsl].rearrange(
                            "(c p) t -> p c t", p=P))
                xt.append(t)
            load_w(wq_sb, wq_d, 0, "wq")
            load_w(wk_sb, wk_d, 0, "wk")
            for h in range(1, HL):
                load_w(wq_sb, wq_d, h, "wq")
                load_w(wk_sb, wk_d, h, "wk")
                load_w(wv_sb, wv_d, h, "wv")
            wu_sb = bigp.tile([P, DL // P, K], bf16)
            nc.sync.dma_start(
                wu_sb[:], wu_d.ap().rearrange("(c p) o -> p c o", p=P))

            out_heads = []

            def emit_unify(qb):
                # partial unify for this 512-token chunk -> DRAM (f32).
                # The very last chunk is drained in halves to shorten the
                # kernel tail.
                for i in range(4):
                    tb = qb * 4 + i
                    psu = ps_mm.tile([P, K], f32, tag="mm")
                    n_acc = 0
                    for hh in range(HL):
                        for f4 in range(4):
                            nc.tensor.matmul(
                                psu[:],
                                out_heads[hh][:, f4, tb * P:(tb + 1) * P],
                                wu_sb[:, hh * 4 + f4, :],
                                start=(n_acc == 0), stop=(n_acc == 15))
                            n_acc += 1
                    u_sb = up.tile([P, K], f32, tag="u")
                    if tb == TC - 1:
                        # drain the very last chunk in halves to shorten
                        # the kernel tail.
                        for half in range(2):
                            hs = slice(half * (K // 2), (half + 1) * (K // 2))
                            nc.vector.tensor_copy(u_sb[:, hs], psu[:, hs])
                            nc.sync.dma_start(
                                out_d[tb * P:(tb + 1) * P, hs], u_sb[:, hs])
                    else:
                        nc.vector.tensor_copy(u_sb[:], psu[:])
                        nc.sync.dma_start(out_d[tb * P:(tb + 1) * P, :],
                                          u_sb[:])

            for h in range(HL):
                # qT: fp8 [P, pair j, qblock, i, tok] with the DoubleRow
                # pair (feature chunks 2j, 2j+1) adjacent -> contiguous
                # [128, 2, 512] moving operand per (j, qb).
                qT = qkvp.tile([P, 2, QB, 2, 512], f8, tag="qT")
                # kT: fp8 [P, pair j, keychunk, i, key] -> contiguous
                # [128, 2, 128] stationary operand per (j, kc16).
                kT = qkvp.tile([P, 2, TC, 2, P], f8, tag="kT")
                # v: bf16, token-major [tok%128, tok//128, feat]
                v = qkvp.tile([P, TC, K], bf16, tag="v")
                if NF8:
                    # v8t: fp8 copy of key-chunks 0..NF8-1 for partial-fp8
                    # attention-output: [P, pair, f4block, i, col].
                    v8t = qkvp.tile([P, NF8 // 2, 4, 2, P], f8, tag="v8")

                # V projection first (wv/x are the first DMAs to land).
                for t16 in range(TC):
                    ps = ps_mm.tile([P, 512], f32, tag="mm")
                    for kc in range(KC):
                        nc.tensor.matmul(
                            ps[:],
                            xt[t16 // 4][:, kc, (t16 % 4) * P:
                                         (t16 % 4 + 1) * P],
                            wv_sb[h][:, kc, :],
                            start=(kc == 0), stop=(kc == KC - 1))
                    nc.scalar.copy(v[:, t16, :], ps[:])
                    if t16 < NF8:
                        nc.vector.tensor_copy(
                            v8t[:, t16 // 2, :, t16 % 2, :],
                            ps[:].rearrange("p (f c) -> p f c", c=P))

                # Q/K projections; fp8 copies on the Vector engine
                # (Scalar stays free for exp + v-copies). PSUM groups
                # alternate between ps_mm and the (idle-at-this-point)
                # ps_o pool for deeper pipelining past copy latency.
                for w_sb, dst in ((wq_sb[h], qT), (wk_sb[h], kT)):
                    for dc in range(KC):
                        for tb in range(QB):
                            g = dc * QB + tb
                            # skip ps_o for the first groups: its tiles'
                            # readers (previous head's normalize muls on
                            # DVE) may still be in flight.
                            pool = ps_o if (g >= 4 and g % 2) else ps_mm
                            ps = pool.tile([P, 512], f32,
                                           tag="o" if pool is ps_o
                                           else "mm")
                            for kc in range(KC):
                                nc.tensor.matmul(
                                    ps[:],
                                    w_sb[:, kc, dc * P:(dc + 1) * P],
                                    xt[tb][:, kc, :],
                                    start=(kc == 0), stop=(kc == KC - 1))
                            if dst is qT:
                                nc.vector.tensor_copy(
                                    qT[:, dc // 2, tb, dc % 2, :], ps[:])
                            else:
                                nc.vector.tensor_copy(
                                    kT[:, dc // 2, tb * 4:(tb + 1) * 4,
                                       dc % 2, :],
                                    ps[:].rearrange("p (c t) -> p c t", t=P))

                out_sbT = outp.tile([P, 4, T], bf16, name=f"out_sbT{h}",
                                    tag=f"oh{h}")
                out_heads.append(out_sbT)

                pending_unify = None
                for qb in range(QB):
                    qsl = slice(qb * 512, (qb + 1) * 512)

                    def scores_group(kc16):
                        # fp8 DoubleRow: 2 MMs contract all 512 features.
                        ps = ps_s.tile([P, 512], f32, tag="s")
                        for j in range(2):
                            nc.tensor.matmul(
                                ps[:],
                                kT[:, j, kc16, :, :],
                                qT[:, j, qb, :, :],
                                start=(j == 0), stop=(j == 1),
                                perf_mode=DR)
                        return ps

                    opsums = [ps_o.tile([P, 512], f32, tag="o",
                                        name=f"opsum{h}_{qb}_{i}")
                              for i in range(4)]
                    sum_acc = attnp.tile([P, 512], bf16, tag="sacc", bufs=3)
                    if NF8:
                        # fp8 exp outputs for chunks 0..NF8-1 [P, pair, i, q]
                        e8 = attnp.tile([P, NF8 // 2, 2, 512], f8,
                                        tag="e8", bufs=2)
                    prev_ps = scores_group(0)
                    if pending_unify is not None:
                        emit_unify(pending_unify)
                        pending_unify = None
                    for kc16 in range(TC):
                        nxt = (scores_group(kc16 + 1)
                               if kc16 + 1 < TC else None)
                        if kc16 < NF8:
                            e_dst = e8[:, kc16 // 2, kc16 % 2, :]
                        else:
                            e_ch = attnp.tile([P, 512], bf16, tag="e",
                                              bufs=4)
                            e_dst = e_ch[:]
                        nc.scalar.activation(e_dst, prev_ps[:], Exp,
                                             bias=0.0,
                                             scale=1.0 / (PRE * PRE))
                        if kc16 == 0:
                            nc.vector.tensor_copy(sum_acc[:], e_dst)
                        else:
                            nc.vector.tensor_add(sum_acc[:], sum_acc[:],
                                                 e_dst)
                        if kc16 < NF8:
                            if kc16 % 2 == 1:
                                pr = kc16 // 2
                                for f4 in range(4):
                                    nc.tensor.matmul(
                                        opsums[f4][:],
                                        v8t[:, pr, f4, :, :],
                                        e8[:, pr, :, :],
                                        start=(kc16 == 1), stop=False,
                                        perf_mode=DR)
                        else:
                            for f4 in range(4):
                                nc.tensor.matmul(
                                    opsums[f4][:],
                                    v[:, kc16, f4 * P:(f4 + 1) * P],
                                    e_ch[:],
                                    start=(NF8 == 0 and kc16 == 0),
                                    stop=(kc16 == TC - 1))
                        prev_ps = nxt
                    # total sums broadcast to all partitions in one matmul
                    # (pool chosen so neither next-qb scores nor next-head
                    # projections wait on the reciprocal's read), then
                    # fast-approx reciprocal and normalize.
                    spool = ps_s if qb == QB - 1 else ps_mm
                    sps = spool.tile([P, 512], f32,
                                     tag="s" if qb == QB - 1 else "mm")
                    nc.tensor.matmul(sps[:], ones[:, :], sum_acc[:],
                                     start=True, stop=True)
                    rinv = attnp.tile([P, 512], f32, tag="rinv", bufs=2)
                    nc.vector.reciprocal_approx_fast(rinv[:], sps[:])
                    for f4 in range(4):
                        nc.vector.tensor_mul(out_sbT[:, f4, qsl],
                                             opsums[f4][:], rinv[:])
                    if h == HL - 1:
                        pending_unify = qb
                if pending_unify is not None:
                    emit_unify(pending_unify)

    nc.compile()
    return nc


def _get_nc():
    if "nc" not in _NC_CACHE:
        _NC_CACHE["nc"] = _build_nc()
    return _NC_CACHE["nc"]


def _make_in_maps(x, Wq, Wk, Wv, Wu, bu):
    inv4 = float(K) ** -0.25
    bf16 = ml_dtypes.bfloat16
    wq_s = (np.asarray(Wq) * (inv4 * PRE)).astype(bf16)
    wk_s = (np.asarray(Wk) * (inv4 * PRE)).astype(bf16)
    wv_s = np.asarray(Wv).astype(bf16)
    wu_s = np.asarray(Wu).astype(bf16)
    in_maps = []
    for c in range(NCORES):
        b, r = c // 2, c % 2
        hs = slice(r * DL, (r + 1) * DL)
        in_maps.append({
            "xT": np.ascontiguousarray(np.asarray(x[b]).T).astype(bf16),
            "wq": np.ascontiguousarray(wq_s[:, hs]),
            "wk": np.ascontiguousarray(wk_s[:, hs]),
            "wv": np.ascontiguousarray(wv_s[:, hs]),
            "wu": np.ascontiguousarray(wu_s[hs, :]),
        })
    return in_maps


def _assemble(results, bu):
    out = np.empty((B, T, K), np.float32)
    bu32 = np.asarray(bu, np.float32)
    for b in range(B):
        out[b] = results[2 * b]["out"] + results[2 * b + 1]["out"] + bu32
    return out


def run_on_hw(x, Wq, Wk, Wv, Wu, bu, trace=False):
    from concourse.bass_utils import run_bass_kernel_spmd
    nc = _get_nc()
    in_maps = _make_in_maps(x, Wq, Wk, Wv, Wu, bu)
    res = run_bass_kernel_spmd(nc, in_maps, core_ids=list(range(NCORES)),
                               trace=trace)
    return _assemble(res.results, bu), res


def kernel(x, Wq, Wk, Wv, Wu, bu):
    out, _ = run_on_hw(x, Wq, Wk, Wv, Wu, bu, trace=False)
    return out
